# revision 47
# baseline (speedup 1.0000x reference)
"""DeepFM forward kernel for 8 Trainium2 NeuronCores (Bass/Tile).

Math (per batch row b):
    lin[b] = x[b] @ w + b0
    C[b]   = sum_k (x[b] @ v)_k^2
    Bq[b]  = sum_f s[f] * x[b,f]^2,   s[f] = sum_k v[f,k]^2
    out[b] = sigmoid(lin[b] + 0.5*C[b] - 0.5*Bq[b])

Data-parallel: batch 16384 sharded 8 ways (2048 rows/core); parameters
replicated. x is shipped pre-transposed (features on partitions) so every
matmul contracts over the partition dim with no on-chip transposes.

Precision scheme (hardware fp32r truncates matmul inputs to 11 mantissa
bits; engine writes to f32r tiles round to the same grid):
  - A-stream (xv + lin): 3 fp32r passes  x11@vw11 + x11@vwl + xl@vw11
    where x11 = round11(x), xl = x - x11 (exact), vw split likewise.
    Residual ~2^-22 relative — fp32-level.
  - B-stream: 1 fp32r pass over m = s*x^2 (ACT Square with per-feature
    sqrt(s) scale, rounded to 11 bits on write). Error ~1e-4 absolute in
    the sigmoid argument.
"""

import numpy as np

import concourse.bass as bass
import concourse.tile as tile
from concourse import bacc, mybir
from concourse.bass_utils import run_bass_kernel_spmd

BATCH, FIELD, EMBED = 16384, 2048, 64
NCORES = 8
BS = BATCH // NCORES   # 2048 batch rows per core
NCHUNK = 512           # psum free-dim per matmul
KTILES = FIELD // 128  # 16 contraction tiles
NCHUNKS = BS // NCHUNK  # 4 batch chunks per core
M = EMBED + 1          # 65 stationary columns: v plus w

F32 = mybir.dt.float32
F32R = mybir.dt.float32r
AF = mybir.ActivationFunctionType

# Two-pass B-stream: adds an exact-residual pass for the quadratic term,
# taking the output to fp32-reference accuracy (~1e-7) at ~10% more time.
PRECISE_B = True


def _build_nc():
    nc = bacc.Bacc("TRN2", target_bir_lowering=False, debug=False)

    xt = nc.declare_dram_parameter("xt", [FIELD, BS], F32, isOutput=False)
    # host-packed SBUF images: [128, KTILES*M], [128, KTILES]
    vw11i = nc.declare_dram_parameter("vw11i", [128, KTILES * M], F32R, isOutput=False)
    vwli = nc.declare_dram_parameter("vwli", [128, KTILES * M], F32R, isOutput=False)
    sqsi = nc.declare_dram_parameter("sqsi", [128, KTILES], F32, isOutput=False)
    red = nc.declare_dram_parameter("red", [97, 1], F32, isOutput=False)
    ones = nc.declare_dram_parameter("ones", [128, 1], F32R, isOutput=False)
    bvec = nc.declare_dram_parameter("bvec", [1, 1], F32, isOutput=False)
    y = nc.declare_dram_parameter("y", [NCHUNKS, NCHUNK], F32, isOutput=True)

    with tile.TileContext(nc) as tc:
        with (
            tc.tile_pool(name="consts", bufs=1) as consts,
            tc.tile_pool(name="xin", bufs=4) as xin,
            tc.tile_pool(name="x11p", bufs=4) as x11p,
            tc.tile_pool(name="xlp", bufs=4) as xlp,
            tc.tile_pool(name="mfp", bufs=3) as mfp,
            tc.tile_pool(name="mrp", bufs=3) as mrp,
            tc.tile_pool(name="mlp", bufs=3) as mlp,
            tc.tile_pool(name="redrhs", bufs=4) as redrhs,
            tc.tile_pool(name="outp", bufs=2) as outp,
            tc.tile_pool(name="psA", bufs=NCHUNKS, space="PSUM") as psA,
            tc.tile_pool(name="psB", bufs=NCHUNKS, space="PSUM") as psB,
        ):
            # ---- replicated parameters, loaded once. Spread across queues
            # so Pool's x11 copies and DVE's subs start as early as possible:
            # vw11 leads the sync queue (first matmul needs it), small consts
            # ride ACT, ones leads Pool. ----
            vw11 = consts.tile([128, KTILES * M], F32R)
            nc.sync.dma_start(vw11[:, :], vw11i[:, :])
            sqs_sb = consts.tile([128, KTILES], F32)
            nc.scalar.dma_start(sqs_sb[:, :], sqsi[:, :])
            vwl = consts.tile([128, KTILES * M], F32R)
            nc.scalar.dma_start(vwl[:, :], vwli[:, :])
            ones_sb = consts.tile([128, 1], F32R)
            nc.gpsimd.dma_start(ones_sb[:, :], ones[:, :])
            red_sb = consts.tile([97, 1], F32)
            nc.scalar.dma_start(red_sb[:, :], red[:, :])
            b_sb = consts.tile([1, 1], F32)
            nc.scalar.dma_start(b_sb[:, :], bvec[:, :])

            psumA = [
                psA.tile([M, NCHUNK], F32, name=f"psumA{n}", tag="psumA")
                for n in range(NCHUNKS)
            ]
            psumB = [
                psB.tile([1, NCHUNK], F32, name=f"psumB{n}", tag="psumB")
                for n in range(NCHUNKS)
            ]

            def process(k, pieces):
                """One contraction stripe k, split into `pieces` column blocks
                (list of (col_lo, col_hi)); each block covers whole chunks."""
                vw11_k = vw11[:, k * M:(k + 1) * M]
                vwl_k = vwl[:, k * M:(k + 1) * M]
                first, last = k == 0, k == KTILES - 1
                for lo, hi in pieces:
                    w = hi - lo
                    xk = xin.tile([128, w], F32, name=f"xk{k}_{lo}", tag="xk")
                    nc.sync.dma_start(xk[:, :], xt[k * 128:(k + 1) * 128, lo:hi])
                    x11 = x11p.tile([128, w], F32R, name=f"x11{k}_{lo}", tag="x11")
                    nc.gpsimd.tensor_copy(x11[:, :], xk[:, :])
                    xl = xlp.tile([128, w], F32R, name=f"xl{k}_{lo}", tag="xl")
                    nc.vector.tensor_sub(xl[:, :], xk[:, :], x11[:, :])
                    if PRECISE_B:
                        # m = s*x^2 in f32; hi-part = round11(m) on Pool;
                        # lo-part = m - hi (exact) on DVE. Both pass the PE
                        # untruncated.
                        mf = mfp.tile([128, w], F32, name=f"mf{k}_{lo}", tag="mf")
                        nc.scalar.activation(
                            mf[:, :], xk[:, :], AF.Square, scale=sqs_sb[:, k:k + 1]
                        )
                        mr = mrp.tile([128, w], F32R, name=f"mr{k}_{lo}", tag="mr")
                        nc.gpsimd.tensor_copy(mr[:, :], mf[:, :])
                        ml = mlp.tile([128, w], F32R, name=f"ml{k}_{lo}", tag="ml")
                        nc.vector.tensor_sub(ml[:, :], mf[:, :], mr[:, :])
                    else:
                        mr = mrp.tile([128, w], F32R, name=f"mr{k}_{lo}", tag="mr")
                        nc.scalar.activation(
                            mr[:, :], xk[:, :], AF.Square, scale=sqs_sb[:, k:k + 1]
                        )
                        ml = None

                    chunks = range(lo // NCHUNK, hi // NCHUNK)
                    # x11-dependent matmuls first (ready earliest), then xl/m
                    for n in chunks:
                        sl = slice(n * NCHUNK - lo, (n + 1) * NCHUNK - lo)
                        nc.tensor.matmul(
                            psumA[n][:, :], vw11_k, x11[:, sl],
                            start=first, stop=False,
                        )
                        nc.tensor.matmul(
                            psumA[n][:, :], vwl_k, x11[:, sl],
                            start=False, stop=False,
                        )
                    for n in chunks:
                        sl = slice(n * NCHUNK - lo, (n + 1) * NCHUNK - lo)
                        nc.tensor.matmul(
                            psumA[n][:, :], vw11_k, xl[:, sl],
                            start=False, stop=last,
                        )
                    for n in chunks:
                        sl = slice(n * NCHUNK - lo, (n + 1) * NCHUNK - lo)
                        nc.tensor.matmul(
                            psumB[n][:, :], ones_sb[:, :], mr[:, sl],
                            start=first, stop=(last and not PRECISE_B),
                        )
                    if PRECISE_B:
                        for n in chunks:
                            sl = slice(n * NCHUNK - lo, (n + 1) * NCHUNK - lo)
                            nc.tensor.matmul(
                                psumB[n][:, :], ones_sb[:, :], ml[:, sl],
                                start=False, stop=last,
                            )

            # first stripe in quarters to fill the pipeline quickly, then whole
            process(0, [(i * NCHUNK, (i + 1) * NCHUNK) for i in range(NCHUNKS)])
            for k in range(1, KTILES):
                process(k, [(0, BS)])

            # ---- epilogue: batch same-function ACT ops to avoid table reloads ----
            rhss, psumCs = [], []
            for n in range(NCHUNKS):
                # rows 0..63 = (xv)^2, 64 = lin, 65..95 zero, 96 = Bq
                rhs = redrhs.tile([97, NCHUNK], F32, name=f"rhs{n}", tag="rhs")
                nc.scalar.activation(rhs[0:EMBED, :], psumA[n][0:EMBED, :], AF.Square)
                nc.gpsimd.memset(rhs[64:96, :], 0.0)
                rhss.append(rhs)
            for n in range(NCHUNKS):
                nc.vector.tensor_copy(rhss[n][64:65, :], psumA[n][EMBED:M, :])
                nc.vector.tensor_copy(rhss[n][96:97, :], psumB[n][:, :])
            for n in range(NCHUNKS):
                # reuse a freed psumA slot (all psumA released after rhs built)
                psumC = psA.tile([1, NCHUNK], F32, name=f"psumC{n}", tag="psumA")
                nc.tensor.matmul(
                    psumC[:, :], red_sb[:, :], rhss[n][:, :], start=True, stop=True
                )
                out_sb = outp.tile([1, NCHUNK], F32, name=f"out{n}", tag="out")
                nc.scalar.activation(
                    out_sb[:, :], psumC[:, :], AF.Sigmoid, bias=b_sb[0:1, 0:1]
                )
                nc.gpsimd.dma_start(y[n:n + 1, :], out_sb[:, :])

    nc.compile()
    return nc


_NC_CACHE = None


def _prep_inputs(x, w, b, v):
    x = np.ascontiguousarray(x, dtype=np.float32)
    w = np.asarray(w, dtype=np.float32).reshape(FIELD, 1)
    v = np.asarray(v, dtype=np.float32)
    b0 = float(np.asarray(b, dtype=np.float32).reshape(-1)[0])

    s64 = (v.astype(np.float64) ** 2).sum(axis=1)
    sqs = np.sqrt(s64).astype(np.float32)
    vw = np.concatenate([v, w], axis=1).astype(np.float32)  # [FIELD, M]

    # hi/lo split on the f32r (11-mantissa-bit) grid; vw11 + vwl == vw to
    # within half an f32 ulp, both pieces pass through the PE unaltered.
    ui = vw.view(np.uint32).astype(np.uint64)
    r = (((ui + (1 << 11)) >> 12) << 12) & 0xFFFFFFFF
    vw11 = r.astype(np.uint32).view(np.float32)
    ui_l = ((vw.astype(np.float64) - vw11).astype(np.float32)
            .view(np.uint32).astype(np.uint64))
    r_l = (((ui_l + (1 << 11)) >> 12) << 12) & 0xFFFFFFFF
    vwl = r_l.astype(np.uint32).view(np.float32)

    def pack(a):  # [FIELD, M] -> [128, KTILES*M] SBUF image
        return np.ascontiguousarray(
            a.reshape(KTILES, 128, M).transpose(1, 0, 2).reshape(128, KTILES * M)
        )

    vw11i, vwli = pack(vw11), pack(vwl)
    sqsi = np.ascontiguousarray(sqs.reshape(KTILES, 128).T)

    red = np.zeros((97, 1), np.float32)
    red[0:EMBED, 0] = 0.5
    red[EMBED, 0] = 1.0
    red[96, 0] = -0.5
    ones = np.ones((128, 1), np.float32)
    bvec = np.full((1, 1), b0, np.float32)

    in_maps = []
    for c in range(NCORES):
        xt_c = np.ascontiguousarray(x[c * BS:(c + 1) * BS, :].T)
        in_maps.append({
            "xt": xt_c, "vw11i": vw11i, "vwli": vwli, "sqsi": sqsi,
            "red": red, "ones": ones, "bvec": bvec,
        })
    return in_maps


def _run(x, w, b, v, **spmd_kwargs):
    global _NC_CACHE
    if _NC_CACHE is None:
        _NC_CACHE = _build_nc()
    nc = _NC_CACHE

    in_maps = _prep_inputs(x, w, b, v)
    res = run_bass_kernel_spmd(nc, in_maps, list(range(NCORES)), **spmd_kwargs)
    out = np.concatenate(
        [res.results[c]["y"].reshape(BS) for c in range(NCORES)]
    )
    return out.reshape(BATCH, 1).astype(np.float32), res


def kernel(x, w, b, v):
    out, _ = _run(x, w, b, v)
    return out


# revision 49
# speedup vs baseline: 1.0227x; 1.0227x over previous
"""DeepFM forward kernel for 8 Trainium2 NeuronCores (Bass/Tile).

Math (per batch row b):
    lin[b] = x[b] @ w + b0
    C[b]   = sum_k (x[b] @ v)_k^2
    Bq[b]  = sum_f s[f] * x[b,f]^2,   s[f] = sum_k v[f,k]^2
    out[b] = sigmoid(lin[b] + 0.5*C[b] - 0.5*Bq[b])

Data-parallel: batch 16384 sharded 8 ways (2048 rows/core); parameters
replicated. x is shipped pre-transposed (features on partitions) so every
matmul contracts over the partition dim with no on-chip transposes.

Precision scheme (hardware fp32r truncates matmul inputs to 11 mantissa
bits; engine writes to f32r tiles round to the same grid):
  - A-stream (xv + lin): 3 fp32r passes  x11@vw11 + x11@vwl + xl@vw11
    where x11 = round11(x), xl = x - x11 (exact), vw split likewise.
    Residual ~2^-22 relative — fp32-level.
  - B-stream (PRECISE_B): 2 fp32r passes over m = s*x^2 (ACT Square with
    per-feature sqrt(s) scale): hi = round11(m) and the exact residual
    m - hi, accumulated into the same PSUM row. End-to-end output error is
    at the fp32 reference's own noise floor (~1e-6 norm rel).
    With PRECISE_B=False: single truncated pass, ~2e-4 absmax, ~15% faster.
"""

import numpy as np

import concourse.bass as bass
import concourse.tile as tile
from concourse import bacc, mybir
from concourse.bass_utils import run_bass_kernel_spmd

BATCH, FIELD, EMBED = 16384, 2048, 64
NCORES = 8
BS = BATCH // NCORES   # 2048 batch rows per core
NCHUNK = 512           # psum free-dim per matmul
KTILES = FIELD // 128  # 16 contraction tiles
NCHUNKS = BS // NCHUNK  # 4 batch chunks per core
M = EMBED + 1          # 65 stationary columns: v plus w

F32 = mybir.dt.float32
F32R = mybir.dt.float32r
AF = mybir.ActivationFunctionType

# Two-pass B-stream: adds an exact-residual pass for the quadratic term,
# taking the output to fp32-reference accuracy (~1e-7) at ~10% more time.
PRECISE_B = True


def _build_nc():
    nc = bacc.Bacc("TRN2", target_bir_lowering=False, debug=False)

    xt = nc.declare_dram_parameter("xt", [FIELD, BS], F32, isOutput=False)
    # host-packed SBUF images: [128, KTILES*M], [128, KTILES]
    vw11i = nc.declare_dram_parameter("vw11i", [128, KTILES * M], F32R, isOutput=False)
    vwli = nc.declare_dram_parameter("vwli", [128, KTILES * M], F32R, isOutput=False)
    sqsi = nc.declare_dram_parameter("sqsi", [128, KTILES], F32, isOutput=False)
    red = nc.declare_dram_parameter("red", [97, 1], F32, isOutput=False)
    ones = nc.declare_dram_parameter("ones", [128, 1], F32R, isOutput=False)
    bvec = nc.declare_dram_parameter("bvec", [1, 1], F32, isOutput=False)
    y = nc.declare_dram_parameter("y", [NCHUNKS, NCHUNK], F32, isOutput=True)

    with tile.TileContext(nc) as tc:
        with (
            tc.tile_pool(name="consts", bufs=1) as consts,
            tc.tile_pool(name="xin", bufs=5) as xin,
            tc.tile_pool(name="x11p", bufs=5) as x11p,
            tc.tile_pool(name="xlp", bufs=4) as xlp,
            tc.tile_pool(name="mfp", bufs=3) as mfp,
            tc.tile_pool(name="mrp", bufs=3) as mrp,
            tc.tile_pool(name="mlp", bufs=3) as mlp,
            tc.tile_pool(name="redrhs", bufs=4) as redrhs,
            tc.tile_pool(name="outp", bufs=2) as outp,
            tc.tile_pool(name="psA", bufs=NCHUNKS, space="PSUM") as psA,
            tc.tile_pool(name="psB", bufs=NCHUNKS, space="PSUM") as psB,
        ):
            # ---- replicated parameters, loaded once. Spread across queues
            # so Pool's x11 copies and DVE's subs start as early as possible:
            # vw11 leads the sync queue (first matmul needs it), small consts
            # ride ACT, ones leads Pool. ----
            vw11 = consts.tile([128, KTILES * M], F32R)
            nc.sync.dma_start(vw11[:, :], vw11i[:, :])
            sqs_sb = consts.tile([128, KTILES], F32)
            nc.scalar.dma_start(sqs_sb[:, :], sqsi[:, :])
            vwl = consts.tile([128, KTILES * M], F32R)
            nc.scalar.dma_start(vwl[:, :], vwli[:, :])
            ones_sb = consts.tile([128, 1], F32R)
            nc.gpsimd.dma_start(ones_sb[:, :], ones[:, :])
            red_sb = consts.tile([97, 1], F32)
            nc.scalar.dma_start(red_sb[:, :], red[:, :])
            b_sb = consts.tile([1, 1], F32)
            nc.scalar.dma_start(b_sb[:, :], bvec[:, :])

            psumA = [
                psA.tile([M, NCHUNK], F32, name=f"psumA{n}", tag="psumA")
                for n in range(NCHUNKS)
            ]
            psumB = [
                psB.tile([1, NCHUNK], F32, name=f"psumB{n}", tag="psumB")
                for n in range(NCHUNKS)
            ]

            def process(k, pieces):
                """One contraction stripe k, split into `pieces` column blocks
                (list of (col_lo, col_hi)); each block covers whole chunks."""
                vw11_k = vw11[:, k * M:(k + 1) * M]
                vwl_k = vwl[:, k * M:(k + 1) * M]
                first, last = k == 0, k == KTILES - 1
                for lo, hi in pieces:
                    w = hi - lo
                    xk = xin.tile([128, w], F32, name=f"xk{k}_{lo}", tag="xk")
                    nc.sync.dma_start(xk[:, :], xt[k * 128:(k + 1) * 128, lo:hi])
                    x11 = x11p.tile([128, w], F32R, name=f"x11{k}_{lo}", tag="x11")
                    nc.gpsimd.tensor_copy(x11[:, :], xk[:, :])
                    xl = xlp.tile([128, w], F32R, name=f"xl{k}_{lo}", tag="xl")
                    nc.vector.tensor_sub(xl[:, :], xk[:, :], x11[:, :])
                    if PRECISE_B:
                        # m = s*x^2 in f32; hi-part = round11(m) on Pool;
                        # lo-part = m - hi (exact) on DVE. Both pass the PE
                        # untruncated.
                        mf = mfp.tile([128, w], F32, name=f"mf{k}_{lo}", tag="mf")
                        nc.scalar.activation(
                            mf[:, :], xk[:, :], AF.Square, scale=sqs_sb[:, k:k + 1]
                        )
                        mr = mrp.tile([128, w], F32R, name=f"mr{k}_{lo}", tag="mr")
                        nc.gpsimd.tensor_copy(mr[:, :], mf[:, :])
                        ml = mlp.tile([128, w], F32R, name=f"ml{k}_{lo}", tag="ml")
                        nc.vector.tensor_sub(ml[:, :], mf[:, :], mr[:, :])
                    else:
                        mr = mrp.tile([128, w], F32R, name=f"mr{k}_{lo}", tag="mr")
                        nc.scalar.activation(
                            mr[:, :], xk[:, :], AF.Square, scale=sqs_sb[:, k:k + 1]
                        )
                        ml = None

                    chunks = range(lo // NCHUNK, hi // NCHUNK)
                    # x11-dependent matmuls first (ready earliest), then xl/m
                    for n in chunks:
                        sl = slice(n * NCHUNK - lo, (n + 1) * NCHUNK - lo)
                        nc.tensor.matmul(
                            psumA[n][:, :], vw11_k, x11[:, sl],
                            start=first, stop=False,
                        )
                        nc.tensor.matmul(
                            psumA[n][:, :], vwl_k, x11[:, sl],
                            start=False, stop=False,
                        )
                    for n in chunks:
                        sl = slice(n * NCHUNK - lo, (n + 1) * NCHUNK - lo)
                        nc.tensor.matmul(
                            psumA[n][:, :], vw11_k, xl[:, sl],
                            start=False, stop=last,
                        )
                    for n in chunks:
                        sl = slice(n * NCHUNK - lo, (n + 1) * NCHUNK - lo)
                        nc.tensor.matmul(
                            psumB[n][:, :], ones_sb[:, :], mr[:, sl],
                            start=first, stop=(last and not PRECISE_B),
                        )
                    if PRECISE_B:
                        for n in chunks:
                            sl = slice(n * NCHUNK - lo, (n + 1) * NCHUNK - lo)
                            nc.tensor.matmul(
                                psumB[n][:, :], ones_sb[:, :], ml[:, sl],
                                start=False, stop=last,
                            )

            # first stripe in quarters to fill the pipeline quickly, then whole
            process(0, [(i * NCHUNK, (i + 1) * NCHUNK) for i in range(NCHUNKS)])
            for k in range(1, KTILES):
                process(k, [(0, BS)])

            # ---- epilogue: batch same-function ACT ops to avoid table reloads ----
            rhss, psumCs = [], []
            for n in range(NCHUNKS):
                # rows 0..63 = (xv)^2, 64 = lin, 65..95 zero, 96 = Bq
                rhs = redrhs.tile([97, NCHUNK], F32, name=f"rhs{n}", tag="rhs")
                nc.scalar.activation(rhs[0:EMBED, :], psumA[n][0:EMBED, :], AF.Square)
                nc.gpsimd.memset(rhs[64:96, :], 0.0)
                rhss.append(rhs)
            for n in range(NCHUNKS):
                nc.vector.tensor_copy(rhss[n][64:65, :], psumA[n][EMBED:M, :])
                nc.vector.tensor_copy(rhss[n][96:97, :], psumB[n][:, :])
            for n in range(NCHUNKS):
                # reuse a freed psumA slot (all psumA released after rhs built)
                psumC = psA.tile([1, NCHUNK], F32, name=f"psumC{n}", tag="psumA")
                nc.tensor.matmul(
                    psumC[:, :], red_sb[:, :], rhss[n][:, :], start=True, stop=True
                )
                out_sb = outp.tile([1, NCHUNK], F32, name=f"out{n}", tag="out")
                nc.scalar.activation(
                    out_sb[:, :], psumC[:, :], AF.Sigmoid, bias=b_sb[0:1, 0:1]
                )
                nc.gpsimd.dma_start(y[n:n + 1, :], out_sb[:, :])

    nc.compile()
    return nc


_NC_CACHE = None


def _prep_inputs(x, w, b, v):
    x = np.ascontiguousarray(x, dtype=np.float32)
    w = np.asarray(w, dtype=np.float32).reshape(FIELD, 1)
    v = np.asarray(v, dtype=np.float32)
    b0 = float(np.asarray(b, dtype=np.float32).reshape(-1)[0])

    s64 = (v.astype(np.float64) ** 2).sum(axis=1)
    sqs = np.sqrt(s64).astype(np.float32)
    vw = np.concatenate([v, w], axis=1).astype(np.float32)  # [FIELD, M]

    # hi/lo split on the f32r (11-mantissa-bit) grid; vw11 + vwl == vw to
    # within half an f32 ulp, both pieces pass through the PE unaltered.
    ui = vw.view(np.uint32).astype(np.uint64)
    r = (((ui + (1 << 11)) >> 12) << 12) & 0xFFFFFFFF
    vw11 = r.astype(np.uint32).view(np.float32)
    ui_l = ((vw.astype(np.float64) - vw11).astype(np.float32)
            .view(np.uint32).astype(np.uint64))
    r_l = (((ui_l + (1 << 11)) >> 12) << 12) & 0xFFFFFFFF
    vwl = r_l.astype(np.uint32).view(np.float32)

    def pack(a):  # [FIELD, M] -> [128, KTILES*M] SBUF image
        return np.ascontiguousarray(
            a.reshape(KTILES, 128, M).transpose(1, 0, 2).reshape(128, KTILES * M)
        )

    vw11i, vwli = pack(vw11), pack(vwl)
    sqsi = np.ascontiguousarray(sqs.reshape(KTILES, 128).T)

    red = np.zeros((97, 1), np.float32)
    red[0:EMBED, 0] = 0.5
    red[EMBED, 0] = 1.0
    red[96, 0] = -0.5
    ones = np.ones((128, 1), np.float32)
    bvec = np.full((1, 1), b0, np.float32)

    in_maps = []
    for c in range(NCORES):
        xt_c = np.ascontiguousarray(x[c * BS:(c + 1) * BS, :].T)
        in_maps.append({
            "xt": xt_c, "vw11i": vw11i, "vwli": vwli, "sqsi": sqsi,
            "red": red, "ones": ones, "bvec": bvec,
        })
    return in_maps


def _run(x, w, b, v, **spmd_kwargs):
    global _NC_CACHE
    if _NC_CACHE is None:
        _NC_CACHE = _build_nc()
    nc = _NC_CACHE

    in_maps = _prep_inputs(x, w, b, v)
    res = run_bass_kernel_spmd(nc, in_maps, list(range(NCORES)), **spmd_kwargs)
    out = np.concatenate(
        [res.results[c]["y"].reshape(BS) for c in range(NCORES)]
    )
    return out.reshape(BATCH, 1).astype(np.float32), res


def kernel(x, w, b, v):
    out, _ = _run(x, w, b, v)
    return out


# revision 59
# speedup vs baseline: 1.0761x; 1.0522x over previous
"""DeepFM forward kernel for 8 Trainium2 NeuronCores (Bass/Tile).

Math (per batch row b):
    lin[b] = x[b] @ w + b0
    C[b]   = sum_k (x[b] @ v)_k^2
    Bq[b]  = sum_f s[f] * x[b,f]^2,   s[f] = sum_k v[f,k]^2
    out[b] = sigmoid(lin[b] + 0.5*C[b] - 0.5*Bq[b])

Data-parallel: batch 16384 sharded 8 ways (2048 rows/core); parameters
replicated. x is shipped pre-transposed (features on partitions) so every
matmul contracts over the partition dim with no on-chip transposes.

Precision scheme (hardware fp32r truncates matmul inputs to 11 mantissa
bits; engine writes to f32r tiles round to the same grid):
  - A-stream (xv + lin): 3 fp32r passes  x11@vw11 + x11@vwl + xl@vw11
    where x11 = round11(x), xl = x - x11 (exact), vw split likewise.
    Residual ~2^-22 relative — fp32-level.
  - B-stream (PRECISE_B): 2 fp32r passes over m = s*x^2 (ACT Square with
    per-feature sqrt(s) scale): hi = round11(m) and the exact residual
    m - hi, accumulated into the same PSUM row. End-to-end output error is
    at the fp32 reference's own noise floor (~1e-6 norm rel).
    With PRECISE_B=False: single truncated pass, ~2e-4 absmax, ~15% faster.
"""

import numpy as np

import concourse.bass as bass
import concourse.tile as tile
from concourse import bacc, mybir
from concourse.bass_utils import run_bass_kernel_spmd

BATCH, FIELD, EMBED = 16384, 2048, 64
NCORES = 8
BS = BATCH // NCORES   # 2048 batch rows per core
NCHUNK = 512           # psum free-dim per matmul
KTILES = FIELD // 128  # 16 contraction tiles
NCHUNKS = BS // NCHUNK  # 4 batch chunks per core
M = EMBED + 1          # 65 stationary columns: v plus w

F32 = mybir.dt.float32
F32R = mybir.dt.float32r
AF = mybir.ActivationFunctionType

# Two-pass B-stream: adds an exact-residual pass for the quadratic term,
# taking the output to fp32-reference accuracy (~1e-7) at ~10% more time.
PRECISE_B = True


def _build_nc():
    nc = bacc.Bacc("TRN2", target_bir_lowering=False, debug=False)

    xt = nc.declare_dram_parameter("xt", [FIELD, BS], F32, isOutput=False)
    # host-packed SBUF images: [128, KTILES*M], [128, KTILES]
    vw11i = nc.declare_dram_parameter("vw11i", [128, KTILES * M], F32R, isOutput=False)
    vwli = nc.declare_dram_parameter("vwli", [128, KTILES * M], F32R, isOutput=False)
    sqsi = nc.declare_dram_parameter("sqsi", [128, KTILES], F32, isOutput=False)
    red = nc.declare_dram_parameter("red", [97, 1], F32, isOutput=False)
    ones = nc.declare_dram_parameter("ones", [128, 1], F32R, isOutput=False)
    bvec = nc.declare_dram_parameter("bvec", [1, 1], F32, isOutput=False)
    y = nc.declare_dram_parameter("y", [NCHUNKS, NCHUNK], F32, isOutput=True)

    with tile.TileContext(nc) as tc:
        with (
            tc.tile_pool(name="consts", bufs=1) as consts,
            tc.tile_pool(name="xin", bufs=5) as xin,
            tc.tile_pool(name="x11p", bufs=5) as x11p,
            tc.tile_pool(name="xlp", bufs=4) as xlp,
            tc.tile_pool(name="mfp", bufs=3) as mfp,
            tc.tile_pool(name="mrp", bufs=3) as mrp,
            tc.tile_pool(name="mlp", bufs=3) as mlp,
            tc.tile_pool(name="redrhs", bufs=4) as redrhs,
            tc.tile_pool(name="outp", bufs=2) as outp,
            tc.tile_pool(name="psA", bufs=NCHUNKS, space="PSUM") as psA,
            tc.tile_pool(name="psB", bufs=NCHUNKS, space="PSUM") as psB,
        ):
            # ---- replicated parameters, loaded once. All consts ride the
            # ACT queue so SP streams x and Pool starts x11 copies at t=0;
            # the ones DMA is issued after the first stripe (see below) so it
            # doesn't block Pool's first x11 copy. ----
            vw11 = consts.tile([128, KTILES * M], F32R)
            nc.gpsimd.dma_start(vw11[:, :], vw11i[:, :])
            sqs_sb = consts.tile([128, KTILES], F32)
            nc.scalar.dma_start(sqs_sb[:, :], sqsi[:, :])
            vwl = consts.tile([128, KTILES * M], F32R)
            nc.scalar.dma_start(vwl[:, :], vwli[:, :])
            ones_sb = consts.tile([128, 1], F32R)
            nc.gpsimd.dma_start(ones_sb[:, :], ones[:, :])
            red_sb = consts.tile([97, 1], F32)
            nc.scalar.dma_start(red_sb[:, :], red[:, :])
            b_sb = consts.tile([1, 1], F32)
            nc.scalar.dma_start(b_sb[:, :], bvec[:, :])

            psumA = [
                psA.tile([M, NCHUNK], F32, name=f"psumA{n}", tag="psumA")
                for n in range(NCHUNKS)
            ]
            psumB = [
                psB.tile([1, NCHUNK], F32, name=f"psumB{n}", tag="psumB")
                for n in range(NCHUNKS)
            ]

            def process(k, pieces):
                """One contraction stripe k, split into `pieces` column blocks
                (list of (col_lo, col_hi)); each block covers whole chunks."""
                vw11_k = vw11[:, k * M:(k + 1) * M]
                vwl_k = vwl[:, k * M:(k + 1) * M]
                first, last = k == 0, k == KTILES - 1
                for lo, hi in pieces:
                    w = hi - lo
                    xk = xin.tile([128, w], F32, name=f"xk{k}_{lo}", tag="xk")
                    nc.sync.dma_start(xk[:, :], xt[k * 128:(k + 1) * 128, lo:hi])
                    # Engine balance: DVE is the busiest engine (the two
                    # full-rate f32 subs); hand a 128-col slice of each sub
                    # to GPSIMD, which has slack.
                    spl = w - 128 if w >= 1024 else w
                    x11 = x11p.tile([128, w], F32R, name=f"x11{k}_{lo}", tag="x11")
                    nc.gpsimd.tensor_copy(x11[:, :], xk[:, :])
                    xl = xlp.tile([128, w], F32R, name=f"xl{k}_{lo}", tag="xl")
                    nc.vector.tensor_sub(xl[:, :spl], xk[:, :spl], x11[:, :spl])
                    if spl < w:
                        nc.gpsimd.tensor_sub(
                            xl[:, spl:], xk[:, spl:], x11[:, spl:]
                        )
                    if PRECISE_B:
                        # m = s*x^2 in f32; hi-part = round11(m) on Pool;
                        # lo-part = m - hi (exact) on DVE. Both pass the PE
                        # untruncated.
                        mf = mfp.tile([128, w], F32, name=f"mf{k}_{lo}", tag="mf")
                        nc.scalar.activation(
                            mf[:, :], xk[:, :], AF.Square, scale=sqs_sb[:, k:k + 1]
                        )
                        mr = mrp.tile([128, w], F32R, name=f"mr{k}_{lo}", tag="mr")
                        nc.gpsimd.tensor_copy(mr[:, :], mf[:, :])
                        ml = mlp.tile([128, w], F32R, name=f"ml{k}_{lo}", tag="ml")
                        nc.vector.tensor_sub(ml[:, :spl], mf[:, :spl], mr[:, :spl])
                        if spl < w:
                            nc.gpsimd.tensor_sub(
                                ml[:, spl:], mf[:, spl:], mr[:, spl:]
                            )
                    else:
                        mr = mrp.tile([128, w], F32R, name=f"mr{k}_{lo}", tag="mr")
                        nc.scalar.activation(
                            mr[:, :], xk[:, :], AF.Square, scale=sqs_sb[:, k:k + 1]
                        )
                        ml = None

                    chunks = range(lo // NCHUNK, hi // NCHUNK)
                    # x11-dependent matmuls first (ready earliest), then xl/m
                    for n in chunks:
                        sl = slice(n * NCHUNK - lo, (n + 1) * NCHUNK - lo)
                        nc.tensor.matmul(
                            psumA[n][:, :], vw11_k, x11[:, sl],
                            start=first, stop=False,
                        )
                        nc.tensor.matmul(
                            psumA[n][:, :], vwl_k, x11[:, sl],
                            start=False, stop=False,
                        )
                    for n in chunks:
                        sl = slice(n * NCHUNK - lo, (n + 1) * NCHUNK - lo)
                        nc.tensor.matmul(
                            psumA[n][:, :], vw11_k, xl[:, sl],
                            start=False, stop=last,
                        )
                    for n in chunks:
                        sl = slice(n * NCHUNK - lo, (n + 1) * NCHUNK - lo)
                        nc.tensor.matmul(
                            psumB[n][:, :], ones_sb[:, :], mr[:, sl],
                            start=first, stop=(last and not PRECISE_B),
                        )
                    if PRECISE_B:
                        for n in chunks:
                            sl = slice(n * NCHUNK - lo, (n + 1) * NCHUNK - lo)
                            nc.tensor.matmul(
                                psumB[n][:, :], ones_sb[:, :], ml[:, sl],
                                start=False, stop=last,
                            )

            # First and last stripes in quarters: the first fills the pipeline
            # quickly; the last lets each chunk close its accumulation (and
            # start its epilogue) without waiting for the whole-stripe subs.
            quarters = [(i * NCHUNK, (i + 1) * NCHUNK) for i in range(NCHUNKS)]
            process(0, quarters)
            for k in range(1, KTILES - 1):
                process(k, [(0, BS)])
            process(KTILES - 1, quarters)

            # ---- epilogue: batch same-function ACT ops to avoid table reloads ----
            rhss, psumCs = [], []
            for n in range(NCHUNKS):
                # rows 0..63 = (xv)^2, 64 = lin, 65..95 zero, 96 = Bq
                rhs = redrhs.tile([97, NCHUNK], F32, name=f"rhs{n}", tag="rhs")
                nc.scalar.activation(rhs[0:EMBED, :], psumA[n][0:EMBED, :], AF.Square)
                nc.gpsimd.memset(rhs[64:96, :], 0.0)
                rhss.append(rhs)
            for n in range(NCHUNKS):
                nc.vector.tensor_copy(rhss[n][64:65, :], psumA[n][EMBED:M, :])
                nc.vector.tensor_copy(rhss[n][96:97, :], psumB[n][:, :])
            for n in range(NCHUNKS):
                # reuse a freed psumA slot (all psumA released after rhs built)
                psumC = psA.tile([1, NCHUNK], F32, name=f"psumC{n}", tag="psumA")
                nc.tensor.matmul(
                    psumC[:, :], red_sb[:, :], rhss[n][:, :], start=True, stop=True
                )
                out_sb = outp.tile([1, NCHUNK], F32, name=f"out{n}", tag="out")
                nc.scalar.activation(
                    out_sb[:, :], psumC[:, :], AF.Sigmoid, bias=b_sb[0:1, 0:1]
                )
                nc.gpsimd.dma_start(y[n:n + 1, :], out_sb[:, :])

    nc.compile()
    return nc


_NC_CACHE = None


def _prep_inputs(x, w, b, v):
    x = np.ascontiguousarray(x, dtype=np.float32)
    w = np.asarray(w, dtype=np.float32).reshape(FIELD, 1)
    v = np.asarray(v, dtype=np.float32)
    b0 = float(np.asarray(b, dtype=np.float32).reshape(-1)[0])

    s64 = (v.astype(np.float64) ** 2).sum(axis=1)
    sqs = np.sqrt(s64).astype(np.float32)
    vw = np.concatenate([v, w], axis=1).astype(np.float32)  # [FIELD, M]

    # hi/lo split on the f32r (11-mantissa-bit) grid; vw11 + vwl == vw to
    # within half an f32 ulp, both pieces pass through the PE unaltered.
    ui = vw.view(np.uint32).astype(np.uint64)
    r = (((ui + (1 << 11)) >> 12) << 12) & 0xFFFFFFFF
    vw11 = r.astype(np.uint32).view(np.float32)
    ui_l = ((vw.astype(np.float64) - vw11).astype(np.float32)
            .view(np.uint32).astype(np.uint64))
    r_l = (((ui_l + (1 << 11)) >> 12) << 12) & 0xFFFFFFFF
    vwl = r_l.astype(np.uint32).view(np.float32)

    def pack(a):  # [FIELD, M] -> [128, KTILES*M] SBUF image
        return np.ascontiguousarray(
            a.reshape(KTILES, 128, M).transpose(1, 0, 2).reshape(128, KTILES * M)
        )

    vw11i, vwli = pack(vw11), pack(vwl)
    sqsi = np.ascontiguousarray(sqs.reshape(KTILES, 128).T)

    red = np.zeros((97, 1), np.float32)
    red[0:EMBED, 0] = 0.5
    red[EMBED, 0] = 1.0
    red[96, 0] = -0.5
    ones = np.ones((128, 1), np.float32)
    bvec = np.full((1, 1), b0, np.float32)

    in_maps = []
    for c in range(NCORES):
        xt_c = np.ascontiguousarray(x[c * BS:(c + 1) * BS, :].T)
        in_maps.append({
            "xt": xt_c, "vw11i": vw11i, "vwli": vwli, "sqsi": sqsi,
            "red": red, "ones": ones, "bvec": bvec,
        })
    return in_maps


def _run(x, w, b, v, **spmd_kwargs):
    global _NC_CACHE
    if _NC_CACHE is None:
        _NC_CACHE = _build_nc()
    nc = _NC_CACHE

    in_maps = _prep_inputs(x, w, b, v)
    res = run_bass_kernel_spmd(nc, in_maps, list(range(NCORES)), **spmd_kwargs)
    out = np.concatenate(
        [res.results[c]["y"].reshape(BS) for c in range(NCORES)]
    )
    return out.reshape(BATCH, 1).astype(np.float32), res


def kernel(x, w, b, v):
    out, _ = _run(x, w, b, v)
    return out


# revision 62
# speedup vs baseline: 1.0853x; 1.0086x over previous
"""DeepFM forward kernel for 8 Trainium2 NeuronCores (Bass/Tile).

Math (per batch row b):
    lin[b] = x[b] @ w + b0
    C[b]   = sum_k (x[b] @ v)_k^2
    Bq[b]  = sum_f s[f] * x[b,f]^2,   s[f] = sum_k v[f,k]^2
    out[b] = sigmoid(lin[b] + 0.5*C[b] - 0.5*Bq[b])

Data-parallel: batch 16384 sharded 8 ways (2048 rows/core); parameters
replicated. x is shipped pre-transposed (features on partitions) so every
matmul contracts over the partition dim with no on-chip transposes.

Precision scheme (hardware fp32r truncates matmul inputs to 11 mantissa
bits; engine writes to f32r tiles round to the same grid):
  - A-stream (xv + lin): 3 fp32r passes  x11@vw11 + x11@vwl + xl@vw11
    where x11 = round11(x), xl = x - x11 (exact), vw split likewise.
    Residual ~2^-22 relative — fp32-level.
  - B-stream (PRECISE_B): 2 fp32r passes over m = s*x^2 (ACT Square with
    per-feature sqrt(s) scale): hi = round11(m) and the exact residual
    m - hi, accumulated into the same PSUM row. End-to-end output error is
    at the fp32 reference's own noise floor (~1e-6 norm rel).
    With PRECISE_B=False: single truncated pass, ~2e-4 absmax, ~15% faster.
"""

import numpy as np

import concourse.bass as bass
import concourse.tile as tile
from concourse import bacc, mybir
from concourse.bass_utils import run_bass_kernel_spmd

BATCH, FIELD, EMBED = 16384, 2048, 64
NCORES = 8
BS = BATCH // NCORES   # 2048 batch rows per core
NCHUNK = 512           # psum free-dim per matmul
KTILES = FIELD // 128  # 16 contraction tiles
NCHUNKS = BS // NCHUNK  # 4 batch chunks per core
M = EMBED + 1          # 65 stationary columns: v plus w

F32 = mybir.dt.float32
F32R = mybir.dt.float32r
AF = mybir.ActivationFunctionType

# Two-pass B-stream: adds an exact-residual pass for the quadratic term,
# taking the output to fp32-reference accuracy (~1e-7) at ~10% more time.
PRECISE_B = True


def _build_nc():
    nc = bacc.Bacc("TRN2", target_bir_lowering=False, debug=False)

    xt = nc.declare_dram_parameter("xt", [FIELD, BS], F32, isOutput=False)
    # host-packed SBUF images: [128, KTILES*M], [128, KTILES]
    vw11i = nc.declare_dram_parameter("vw11i", [128, KTILES * M], F32R, isOutput=False)
    vwli = nc.declare_dram_parameter("vwli", [128, KTILES * M], F32R, isOutput=False)
    sqsi = nc.declare_dram_parameter("sqsi", [128, KTILES], F32, isOutput=False)
    red = nc.declare_dram_parameter("red", [97, 1], F32, isOutput=False)
    ones = nc.declare_dram_parameter("ones", [128, 1], F32R, isOutput=False)
    bvec = nc.declare_dram_parameter("bvec", [1, 1], F32, isOutput=False)
    y = nc.declare_dram_parameter("y", [NCHUNKS, NCHUNK], F32, isOutput=True)

    with tile.TileContext(nc) as tc:
        with (
            tc.tile_pool(name="consts", bufs=1) as consts,
            tc.tile_pool(name="xin", bufs=5) as xin,
            tc.tile_pool(name="x11p", bufs=5) as x11p,
            tc.tile_pool(name="xlp", bufs=4) as xlp,
            tc.tile_pool(name="mfp", bufs=3) as mfp,
            tc.tile_pool(name="mrp", bufs=3) as mrp,
            tc.tile_pool(name="mlp", bufs=3) as mlp,
            tc.tile_pool(name="redrhs", bufs=4) as redrhs,
            tc.tile_pool(name="outp", bufs=2) as outp,
            tc.tile_pool(name="psA", bufs=NCHUNKS, space="PSUM") as psA,
            tc.tile_pool(name="psB", bufs=NCHUNKS, space="PSUM") as psB,
        ):
            # ---- replicated parameters, loaded once. All consts ride the
            # ACT queue so SP streams x and Pool starts x11 copies at t=0;
            # the ones DMA is issued after the first stripe (see below) so it
            # doesn't block Pool's first x11 copy. ----
            vw11 = consts.tile([128, KTILES * M], F32R)
            nc.gpsimd.dma_start(vw11[:, :], vw11i[:, :])
            sqs_sb = consts.tile([128, KTILES], F32)
            nc.scalar.dma_start(sqs_sb[:, :], sqsi[:, :])
            vwl = consts.tile([128, KTILES * M], F32R)
            nc.scalar.dma_start(vwl[:, :], vwli[:, :])
            ones_sb = consts.tile([128, 1], F32R)
            nc.gpsimd.dma_start(ones_sb[:, :], ones[:, :])
            red_sb = consts.tile([97, 1], F32)
            nc.scalar.dma_start(red_sb[:, :], red[:, :])
            b_sb = consts.tile([1, 1], F32)
            nc.scalar.dma_start(b_sb[:, :], bvec[:, :])

            psumA = [
                psA.tile([M, NCHUNK], F32, name=f"psumA{n}", tag="psumA")
                for n in range(NCHUNKS)
            ]
            psumB = [
                psB.tile([1, NCHUNK], F32, name=f"psumB{n}", tag="psumB")
                for n in range(NCHUNKS)
            ]

            def process(k, pieces):
                """One contraction stripe k, split into `pieces` column blocks
                (list of (col_lo, col_hi)); each block covers whole chunks."""
                vw11_k = vw11[:, k * M:(k + 1) * M]
                vwl_k = vwl[:, k * M:(k + 1) * M]
                first, last = k == 0, k == KTILES - 1
                for lo, hi in pieces:
                    w = hi - lo
                    xk = xin.tile([128, w], F32, name=f"xk{k}_{lo}", tag="xk")
                    nc.sync.dma_start(xk[:, :], xt[k * 128:(k + 1) * 128, lo:hi])
                    # Engine balance: DVE is the busiest engine (the two
                    # full-rate f32 subs); hand a 128-col slice of each sub
                    # to GPSIMD, which has slack.
                    spl = w - 256 if w >= 1024 else w
                    x11 = x11p.tile([128, w], F32R, name=f"x11{k}_{lo}", tag="x11")
                    nc.gpsimd.tensor_copy(x11[:, :], xk[:, :])
                    xl = xlp.tile([128, w], F32R, name=f"xl{k}_{lo}", tag="xl")
                    nc.vector.tensor_sub(xl[:, :spl], xk[:, :spl], x11[:, :spl])
                    if spl < w:
                        nc.gpsimd.tensor_sub(
                            xl[:, spl:], xk[:, spl:], x11[:, spl:]
                        )
                    if PRECISE_B:
                        # m = s*x^2 in f32; hi-part = round11(m) on Pool;
                        # lo-part = m - hi (exact) on DVE. Both pass the PE
                        # untruncated.
                        mf = mfp.tile([128, w], F32, name=f"mf{k}_{lo}", tag="mf")
                        nc.scalar.activation(
                            mf[:, :], xk[:, :], AF.Square, scale=sqs_sb[:, k:k + 1]
                        )
                        mr = mrp.tile([128, w], F32R, name=f"mr{k}_{lo}", tag="mr")
                        nc.gpsimd.tensor_copy(mr[:, :], mf[:, :])
                        ml = mlp.tile([128, w], F32R, name=f"ml{k}_{lo}", tag="ml")
                        nc.vector.tensor_sub(ml[:, :spl], mf[:, :spl], mr[:, :spl])
                        if spl < w:
                            nc.gpsimd.tensor_sub(
                                ml[:, spl:], mf[:, spl:], mr[:, spl:]
                            )
                    else:
                        mr = mrp.tile([128, w], F32R, name=f"mr{k}_{lo}", tag="mr")
                        nc.scalar.activation(
                            mr[:, :], xk[:, :], AF.Square, scale=sqs_sb[:, k:k + 1]
                        )
                        ml = None

                    chunks = range(lo // NCHUNK, hi // NCHUNK)
                    # x11-dependent matmuls first (ready earliest), then xl/m
                    for n in chunks:
                        sl = slice(n * NCHUNK - lo, (n + 1) * NCHUNK - lo)
                        nc.tensor.matmul(
                            psumA[n][:, :], vw11_k, x11[:, sl],
                            start=first, stop=False,
                        )
                        nc.tensor.matmul(
                            psumA[n][:, :], vwl_k, x11[:, sl],
                            start=False, stop=False,
                        )
                    for n in chunks:
                        sl = slice(n * NCHUNK - lo, (n + 1) * NCHUNK - lo)
                        nc.tensor.matmul(
                            psumA[n][:, :], vw11_k, xl[:, sl],
                            start=False, stop=last,
                        )
                    for n in chunks:
                        sl = slice(n * NCHUNK - lo, (n + 1) * NCHUNK - lo)
                        nc.tensor.matmul(
                            psumB[n][:, :], ones_sb[:, :], mr[:, sl],
                            start=first, stop=(last and not PRECISE_B),
                        )
                    if PRECISE_B:
                        for n in chunks:
                            sl = slice(n * NCHUNK - lo, (n + 1) * NCHUNK - lo)
                            nc.tensor.matmul(
                                psumB[n][:, :], ones_sb[:, :], ml[:, sl],
                                start=False, stop=last,
                            )

            # First and last stripes in quarters: the first fills the pipeline
            # quickly; the last lets each chunk close its accumulation (and
            # start its epilogue) without waiting for the whole-stripe subs.
            quarters = [(i * NCHUNK, (i + 1) * NCHUNK) for i in range(NCHUNKS)]
            process(0, quarters)
            for k in range(1, KTILES - 1):
                process(k, [(0, BS)])
            process(KTILES - 1, quarters)

            # ---- epilogue: batch same-function ACT ops to avoid table reloads ----
            rhss, psumCs = [], []
            for n in range(NCHUNKS):
                # rows 0..63 = (xv)^2, 64 = lin, 65..95 zero, 96 = Bq
                rhs = redrhs.tile([97, NCHUNK], F32, name=f"rhs{n}", tag="rhs")
                nc.scalar.activation(rhs[0:EMBED, :], psumA[n][0:EMBED, :], AF.Square)
                nc.gpsimd.memset(rhs[64:96, :], 0.0)
                rhss.append(rhs)
            for n in range(NCHUNKS):
                nc.vector.tensor_copy(rhss[n][64:65, :], psumA[n][EMBED:M, :])
                nc.vector.tensor_copy(rhss[n][96:97, :], psumB[n][:, :])
            for n in range(NCHUNKS):
                # reuse a freed psumA slot (all psumA released after rhs built)
                psumC = psA.tile([1, NCHUNK], F32, name=f"psumC{n}", tag="psumA")
                nc.tensor.matmul(
                    psumC[:, :], red_sb[:, :], rhss[n][:, :], start=True, stop=True
                )
                out_sb = outp.tile([1, NCHUNK], F32, name=f"out{n}", tag="out")
                nc.scalar.activation(
                    out_sb[:, :], psumC[:, :], AF.Sigmoid, bias=b_sb[0:1, 0:1]
                )
                nc.gpsimd.dma_start(y[n:n + 1, :], out_sb[:, :])

    nc.compile()
    return nc


_NC_CACHE = None


def _prep_inputs(x, w, b, v):
    x = np.ascontiguousarray(x, dtype=np.float32)
    w = np.asarray(w, dtype=np.float32).reshape(FIELD, 1)
    v = np.asarray(v, dtype=np.float32)
    b0 = float(np.asarray(b, dtype=np.float32).reshape(-1)[0])

    s64 = (v.astype(np.float64) ** 2).sum(axis=1)
    sqs = np.sqrt(s64).astype(np.float32)
    vw = np.concatenate([v, w], axis=1).astype(np.float32)  # [FIELD, M]

    # hi/lo split on the f32r (11-mantissa-bit) grid; vw11 + vwl == vw to
    # within half an f32 ulp, both pieces pass through the PE unaltered.
    ui = vw.view(np.uint32).astype(np.uint64)
    r = (((ui + (1 << 11)) >> 12) << 12) & 0xFFFFFFFF
    vw11 = r.astype(np.uint32).view(np.float32)
    ui_l = ((vw.astype(np.float64) - vw11).astype(np.float32)
            .view(np.uint32).astype(np.uint64))
    r_l = (((ui_l + (1 << 11)) >> 12) << 12) & 0xFFFFFFFF
    vwl = r_l.astype(np.uint32).view(np.float32)

    def pack(a):  # [FIELD, M] -> [128, KTILES*M] SBUF image
        return np.ascontiguousarray(
            a.reshape(KTILES, 128, M).transpose(1, 0, 2).reshape(128, KTILES * M)
        )

    vw11i, vwli = pack(vw11), pack(vwl)
    sqsi = np.ascontiguousarray(sqs.reshape(KTILES, 128).T)

    red = np.zeros((97, 1), np.float32)
    red[0:EMBED, 0] = 0.5
    red[EMBED, 0] = 1.0
    red[96, 0] = -0.5
    ones = np.ones((128, 1), np.float32)
    bvec = np.full((1, 1), b0, np.float32)

    in_maps = []
    for c in range(NCORES):
        xt_c = np.ascontiguousarray(x[c * BS:(c + 1) * BS, :].T)
        in_maps.append({
            "xt": xt_c, "vw11i": vw11i, "vwli": vwli, "sqsi": sqsi,
            "red": red, "ones": ones, "bvec": bvec,
        })
    return in_maps


def _run(x, w, b, v, **spmd_kwargs):
    global _NC_CACHE
    if _NC_CACHE is None:
        _NC_CACHE = _build_nc()
    nc = _NC_CACHE

    in_maps = _prep_inputs(x, w, b, v)
    res = run_bass_kernel_spmd(nc, in_maps, list(range(NCORES)), **spmd_kwargs)
    out = np.concatenate(
        [res.results[c]["y"].reshape(BS) for c in range(NCORES)]
    )
    return out.reshape(BATCH, 1).astype(np.float32), res


def kernel(x, w, b, v):
    out, _ = _run(x, w, b, v)
    return out
